# revision 1
# baseline (speedup 1.0000x reference)
"""BatchGRU TRN2 kernel v2: flipped matmul operands.

State/msg chunks are the stationary operand (K<=101, M=128 molecules), weights
stream as rhs (N=300 gate-chunks, f32r full rate). Gate pre-activations land in
natural [128-mol, 300] layout; biases ride an extra ones-row on K (rows 96:128
pre-filled so the write is 32-aligned). h_new is produced in natural layout
(y store is a plain DMA) and PE-transposed back into the stationary state.
"""
import numpy as np
from contextlib import ExitStack

try:
    import jax as _jax
    _jax.config.update("jax_compilation_cache_dir", "/root/problem/jax_cache")
    _jax.config.update("jax_persistent_cache_min_compile_time_secs", 10.0)
    _jax.config.update("jax_persistent_cache_min_entry_size_bytes", 0)
except Exception:
    pass

import concourse.bacc as bacc
import concourse.tile as tile
from concourse import mybir
from concourse.bass_utils import run_bass_kernel_spmd

f32 = mybir.dt.float32
f32r = mybir.dt.float32r
AF = mybir.ActivationFunctionType
ALU = mybir.AluOpType

H = 300
HC = 100
NK = 3
G = 3 * H
L = 48
B = 2048
NCORES = 8
BLOC = B // NCORES
NLOC = BLOC * L
RT = NLOC // 128
NMC = 2                      # molecule chunks of 128 per core

_cached = {}


def build_program():
    if "nc" in _cached:
        return _cached["nc"]
    nc = bacc.Bacc("TRN2", target_bir_lowering=False, debug=False,
                   dynamic_dma_scratch_size=512)

    x_d = nc.declare_dram_parameter("x", [NLOC, H], f32, isOutput=False)
    # weights [k, p(101), g]: row 100 of k=0 carries the bias row
    wx_f_d = nc.declare_dram_parameter("wx_f", [NK, HC + 1, G], f32, isOutput=False)
    wh_f_d = nc.declare_dram_parameter("wh_f", [NK, HC + 1, G], f32, isOutput=False)
    wx_b_d = nc.declare_dram_parameter("wx_b", [NK, HC + 1, G], f32, isOutput=False)
    wh_b_d = nc.declare_dram_parameter("wh_b", [NK, HC + 1, G], f32, isOutput=False)
    brelu_d = nc.declare_dram_parameter("brelu", [HC, NK], f32, isOutput=False)
    y_d = nc.declare_dram_parameter("y", [NLOC, 2 * H], f32, isOutput=True)

    y_r = y_d[:].rearrange("(m l) c -> m l c", l=L)  # [256, 48, 600]

    with tile.TileContext(nc) as tc:
        with ExitStack() as ctx:
            consts = ctx.enter_context(tc.tile_pool(name="consts", bufs=1))

            # xT chunk 0 is [128, NLOC] so rows 96:128 can hold the ones row
            # (row 100 is the one consumed by k=0 matmuls); chunks 1,2 are [100,...]
            xT = [consts.tile([128 if k == 0 else HC, NLOC], f32r, name=f"xT{k}")
                  for k in range(NK)]
            wx_r = {d: consts.tile([HC + 1, NK, G], f32r, name=f"wx_r_{d}") for d in "fb"}
            wh_r = {d: consts.tile([HC + 1, NK, G], f32r, name=f"wh_r_{d}") for d in "fb"}
            brelu_sb = consts.tile([HC, NK], f32)
            id_f32 = consts.tile([128, 128], f32)
            id128r = consts.tile([128, 128], f32r)
            id100r = consts.tile([HC, HC], f32r)
            # stationary (transposed) state, chunk0 rows 96:128 hold ones row
            state = {d: consts.tile([128, NK, BLOC], f32r, name=f"state_{d}")
                     for d in "fb"}
            # natural-layout state copy (also the y output rows)
            hnat = {d: [consts.tile([128, H], f32r, name=f"hnat_{d}{mc}")
                        for mc in range(NMC)] for d in "fb"}

            ones_c = consts.tile([128, 1], f32)
            nc.vector.memset(ones_c, 1.0)

            # ---- identities ----
            with tc.tile_pool(name="idp", bufs=1) as idp:
                rowi = idp.tile([128, 1], mybir.dt.int32)
                coli = idp.tile([128, 128], mybir.dt.int32)
                nc.gpsimd.iota(rowi, pattern=[[0, 1]], base=0, channel_multiplier=1)
                nc.gpsimd.iota(coli, pattern=[[1, 128]], base=0, channel_multiplier=0)
                rowf = idp.tile([128, 1], f32)
                colf = idp.tile([128, 128], f32)
                nc.vector.tensor_copy(out=rowf, in_=rowi)
                nc.vector.tensor_copy(out=colf, in_=coli)
                nc.vector.tensor_scalar(out=id_f32, in0=colf, scalar1=rowf,
                                        scalar2=None, op0=ALU.is_equal)
                nc.scalar.activation(out=id128r, in_=id_f32, func=AF.Copy)
                nc.scalar.activation(out=id100r, in_=id_f32[:HC, :HC], func=AF.Copy)

            nc.sync.dma_start(out=brelu_sb, in_=brelu_d[:])

            # ones rows (32-aligned writes; later [0:100] writers overwrite 96:99)
            nc.scalar.activation(out=xT[0][96:128, :],
                                 in_=ones_c[96:128, 0:1].to_broadcast((32, NLOC)),
                                 func=AF.Copy)
            for d in "fb":
                nc.scalar.activation(
                    out=state[d][96:128, 0, :],
                    in_=ones_c[96:128, 0:1].to_broadcast((32, BLOC)),
                    func=AF.Copy)

            # ---- weights: DMA staging -> ACT cast to f32r ----
            with tc.tile_pool(name="wstage", bufs=2) as wstage:
                for d in "fb":
                    wx_dram = wx_f_d if d == "f" else wx_b_d
                    wh_dram = wh_f_d if d == "f" else wh_b_d
                    st_x = wstage.tile([HC + 1, NK, G], f32, tag="ws", name=f"stx_{d}")
                    nc.sync.dma_start(out=st_x, in_=wx_dram[:].rearrange("k p g -> p k g"))
                    nc.scalar.activation(out=wx_r[d], in_=st_x, func=AF.Copy)
                    st_h = wstage.tile([HC + 1, NK, G], f32, tag="ws", name=f"sth_{d}")
                    nc.sync.dma_start(out=st_h, in_=wh_dram[:].rearrange("k p g -> p k g"))
                    nc.scalar.activation(out=wh_r[d], in_=st_h, func=AF.Copy)

            # ---- prologue: x -> xT, h0, relu ----
            with tc.tile_pool(name="xstage", bufs=4) as xstage, \
                 tc.tile_pool(name="tps0", bufs=4, space="PSUM") as tps0:
                for rt in range(RT):
                    x_nat = xstage.tile([128, H], f32, tag="xn")
                    nc.sync.dma_start(out=x_nat, in_=x_d[rt * 128:(rt + 1) * 128, :])
                    for k in range(NK):
                        ps = tps0.tile([HC, 128], f32, tag="tp")
                        nc.tensor.transpose(out=ps, in_=x_nat[:, k * HC:(k + 1) * HC],
                                            identity=id_f32)
                        nc.scalar.activation(out=xT[k][0:HC, rt * 128:(rt + 1) * 128],
                                             in_=ps, func=AF.Copy)

                # h0 (raw x) into stationary fwd state, copy to bwd
                for k in range(NK):
                    nc.vector.tensor_reduce(
                        out=state["f"][0:HC, k, :],
                        in_=xT[k][0:HC, :].bitcast(f32).rearrange(
                            "p (m l) -> p m l", l=L),
                        axis=mybir.AxisListType.X, op=ALU.max)
                nc.vector.tensor_copy(out=state["b"][0:HC, :, :],
                                      in_=state["f"][0:HC, :, :].bitcast(f32))
                # natural-layout h0 via PE transposes
                for mc in range(NMC):
                    hps = tps0.tile([128, NK, HC], f32r, tag="h0t", name=f"h0t{mc}")
                    for j in range(NK):
                        nc.tensor.transpose(
                            out=hps[:, j, :],
                            in_=state["f"][0:HC, j, mc * 128:(mc + 1) * 128],
                            identity=id100r)
                    nc.scalar.activation(
                        out=hnat["f"][mc],
                        in_=hps.bitcast(f32).rearrange("p a b -> p (a b)"),
                        func=AF.Copy)
                    nc.vector.tensor_copy(out=hnat["b"][mc],
                                          in_=hnat["f"][mc].bitcast(f32))

            # relu(x + bias) in place on xT rows 0:100
            for k in range(NK):
                nc.scalar.activation(out=xT[k][0:HC, :],
                                     in_=xT[k][0:HC, :].bitcast(f32), func=AF.Relu,
                                     bias=brelu_sb[:, k:k + 1], scale=1.0)

            xT_ml = [xT[k].rearrange("p (m l) -> p m l", l=L) for k in range(NK)]

            # ---- recurrence pools ----
            rz_pool = ctx.enter_context(tc.tile_pool(name="rzp", bufs=2, space="PSUM"))
            nn_pool = ctx.enter_context(tc.tile_pool(name="nnp", bufs=1, space="PSUM"))
            tp_pool = ctx.enter_context(tc.tile_pool(name="tpp", bufs=2, space="PSUM"))
            gates = ctx.enter_context(tc.tile_pool(name="gates", bufs=2))

            GATE_N = [(0, 0), (H, 1)]  # (gate col offset, rz slot)

            for s in range(L):
                for d in "fb":
                    t = s if d == "f" else L - 1 - s
                    wx, wh, st = wx_r[d], wh_r[d], state[d]
                    dcol = 0 if d == "f" else 1
                    rzs, nns = [], []
                    for mc in range(NMC):
                        msl = slice(mc * 128, (mc + 1) * 128)
                        rz_ps = rz_pool.tile([128, 2, 512], f32, tag="rz",
                                             name=f"rz_{d}{s}{mc}")
                        nn_ps = nn_pool.tile([128, 2, 512], f32, tag="nn",
                                             name=f"nn_{d}{s}{mc}")
                        rzs.append(rz_ps)
                        nns.append(nn_ps)
                        # r and z gates: x-side + h-side, k0 carries bias row
                        for goff, slot in GATE_N:
                            gs = slice(goff, goff + H)
                            for k in range(NK):
                                kp = HC + 1 if k == 0 else HC
                                nc.tensor.matmul(
                                    out=rz_ps[:, slot, 0:H],
                                    lhsT=xT_ml[k][0:kp, msl, t],
                                    rhs=wx[0:kp, k, gs],
                                    start=(k == 0), stop=False)
                            for k in range(NK):
                                kp = HC + 1 if k == 0 else HC
                                nc.tensor.matmul(
                                    out=rz_ps[:, slot, 0:H],
                                    lhsT=st[0:kp, k, msl],
                                    rhs=wh[0:kp, k, gs],
                                    start=False, stop=(k == NK - 1))
                        # n gate: xn into slot 0, hn into slot 1
                        ngs = slice(2 * H, 3 * H)
                        for k in range(NK):
                            kp = HC + 1 if k == 0 else HC
                            nc.tensor.matmul(
                                out=nn_ps[:, 0, 0:H],
                                lhsT=xT_ml[k][0:kp, msl, t],
                                rhs=wx[0:kp, k, ngs],
                                start=(k == 0), stop=(k == NK - 1))
                        for k in range(NK):
                            kp = HC + 1 if k == 0 else HC
                            nc.tensor.matmul(
                                out=nn_ps[:, 1, 0:H],
                                lhsT=st[0:kp, k, msl],
                                rhs=wh[0:kp, k, ngs],
                                start=(k == 0), stop=(k == NK - 1))

                    for mc in range(NMC):
                        msl = slice(mc * 128, (mc + 1) * 128)
                        rz_ps, nn_ps = rzs[mc], nns[mc]
                        hn_ = hnat[d][mc]
                        r_sb = gates.tile([128, H], f32, tag="rs", name=f"rs_{d}{s}{mc}")
                        z_sb = gates.tile([128, H], f32, tag="zs", name=f"zs_{d}{s}{mc}")
                        t1 = gates.tile([128, H], f32, tag="t1", name=f"t1_{d}{s}{mc}")
                        n_sb = gates.tile([128, H], f32, tag="ns", name=f"ns_{d}{s}{mc}")
                        nc.scalar.activation(out=r_sb, in_=rz_ps[:, 0, 0:H],
                                             func=AF.Sigmoid)
                        nc.scalar.activation(out=z_sb, in_=rz_ps[:, 1, 0:H],
                                             func=AF.Sigmoid)
                        nc.vector.tensor_mul(out=t1, in0=r_sb, in1=nn_ps[:, 1, 0:H])
                        nc.vector.tensor_add(out=t1, in0=t1, in1=nn_ps[:, 0, 0:H])
                        nc.scalar.activation(out=n_sb, in_=t1, func=AF.Tanh)
                        # t1 = h_old - n (gpsimd, sbuf only)
                        nc.gpsimd.tensor_sub(out=t1, in0=hn_.bitcast(f32), in1=n_sb)
                        nc.vector.tensor_mul(out=t1, in0=z_sb, in1=t1)
                        # h_new (natural) = n + z*(h_old - n)
                        nc.vector.tensor_add(out=hn_, in0=n_sb, in1=t1)

                        # y store straight from natural h_new
                        nc.sync.dma_start(
                            out=y_r[msl, t, dcol * H:(dcol + 1) * H],
                            in_=hn_.bitcast(f32))

                        # transpose h_new back into the stationary state
                        hps = tp_pool.tile([HC, NK, 128], f32r, tag="tp",
                                           name=f"tp_{d}{s}{mc}")
                        for j in range(NK):
                            nc.tensor.transpose(
                                out=hps[:, j, :],
                                in_=hn_[:, j * HC:(j + 1) * HC],
                                identity=id128r)
                        for j in range(NK):
                            nc.scalar.activation(out=st[0:HC, j, msl],
                                                 in_=hps[:, j, :].bitcast(f32),
                                                 func=AF.Copy)

    nc.compile()
    _cached["nc"] = nc
    return nc


def _prep_shared_inputs(bias, w_ih_f, w_hh_f, b_ih_f, b_hh_f,
                        w_ih_b, w_hh_b, b_ih_b, b_hh_b):
    def pack_wx(w_ih, b_ih):
        out = np.zeros((NK, HC + 1, G), np.float32)
        out[:, :HC, :] = np.asarray(w_ih, np.float32).T.reshape(NK, HC, G)
        out[0, HC, 2 * H:] = np.asarray(b_ih, np.float32)[2 * H:]  # xn bias
        return out

    def pack_wh(w_hh, b_ih, b_hh):
        out = np.zeros((NK, HC + 1, G), np.float32)
        out[:, :HC, :] = np.asarray(w_hh, np.float32).T.reshape(NK, HC, G)
        brow = np.asarray(b_hh, np.float64).copy()
        brow[:2 * H] += np.asarray(b_ih, np.float64)[:2 * H]
        out[0, HC, :] = brow.astype(np.float32)
        return out

    return {
        "wx_f": pack_wx(w_ih_f, b_ih_f), "wh_f": pack_wh(w_hh_f, b_ih_f, b_hh_f),
        "wx_b": pack_wx(w_ih_b, b_ih_b), "wh_b": pack_wh(w_hh_b, b_ih_b, b_hh_b),
        "brelu": np.ascontiguousarray(
            np.asarray(bias, np.float32).reshape(NK, HC).T),
    }


def _run(in_maps, trace=False, **kw):
    nc = build_program()
    return run_bass_kernel_spmd(nc, in_maps, list(range(NCORES)), trace=trace, **kw)


def kernel(x, batch, num_moles, max_len, bias, w_ih_f, w_hh_f, b_ih_f, b_hh_f,
           w_ih_b, w_hh_b, b_ih_b, b_hh_b):
    x = np.asarray(x, np.float32)
    batch = np.asarray(batch)
    assert int(num_moles) == B and int(max_len) == L
    assert x.shape == (B * L, H)
    expected_batch = np.repeat(np.arange(B, dtype=batch.dtype), L)
    assert np.array_equal(batch, expected_batch), \
        "kernel assumes uniform 48-length molecules"

    shared = _prep_shared_inputs(
        bias, w_ih_f, w_hh_f, b_ih_f, b_hh_f,
        w_ih_b, w_hh_b, b_ih_b, b_hh_b)

    in_maps = [dict(shared, x=np.ascontiguousarray(x[c * NLOC:(c + 1) * NLOC]))
               for c in range(NCORES)]
    res = _run(in_maps).results
    return np.concatenate([res[c]["y"] for c in range(NCORES)], axis=0)

